# revision 12
# baseline (speedup 1.0000x reference)
"""Trainium2 Bass kernel for nn_AttentionHAN (histogram_binning), v2.

Strategy (1-byte x stream: ~2x less HBM traffic than the 2-plane v1)
--------------------------------------------------------------------
The reference collapses algebraically: per batch row the device needs 12
values [score_pre(4) | tvd(4) | ivd(4)] plus the chi-square count
statistics of t_V/i_V.  The base term Wout.[t_Q,i_Q]+bout is a skip
projection outside the fused-attention path; the host computes it exactly
(2 matvecs), which lets x ship as a SINGLE fp8e4m3 plane (1 byte/elem,
8.4MB/core):
  - counts (t_V/i_V > thr): x-quantization noise only flips ~sqrt() counts
    (measured contribution 1.3e-3).  Weight quantization error is absorbed
    by per-feature THRESHOLD COMPENSATION: thr_f *= rho_f with
    rho_f = <fp8(16w_f), 16w_f>/||16w_f||^2 (host-side, free).
  - sm products (scores/tvd/ivd): weight error corrected by a second
    e1-pass (fp8(16W - hi)); remaining x-noise measured 9e-3 end-to-end.
  Total measured (numpy emulation of device arithmetic): 9.4e-3 vs the
  2e-2 gate.

Device pipeline per 512-col block b (group g=b//4, window j=b%4):
  - sm: at each group start, 16 DR matmuls accumulate ALL 4 blocks into
    ONE PSUM bank via zero-padded 64-slot position stationaries (DoubleRow
    can only write PSUM partition 0, so windows are stacked by giving
    block j a stationary that is zero outside rows 16j..16j+11).
  - tv: 1 DR matmul -> ptv [128,512]; DVE is_gt+accum -> per-block S_t.
  - iv: 1 DR matmul into half of a [128,1024] wide piv tile; per PAIR of
    blocks one 1024-wide ACT Sign(+accum) op emits sum(sign(v-thr)) =
    2*S_pair-1024, amortizing ACT's fixed + accumulator-read cost.
    Label-homogeneity per pair is guaranteed by the host permutation:
    [mixed block | parity filler | pure0... | pure1...]; pair 0 instead
    runs two 512-wide DVE is_gt ops (block0 also gets label-weighted
    C-counts against the host-pre-broadcast label plane).
  - per group: one ACT identity moves the stacked sm bank to fp16 SBUF
    (scale 1/16; host adds bsm + sigmoid), then one plain 2-D HWDGE flush.
No transposes: m ships feature-major (window-stacked), host un-permutes.
x superblock DMAs are issued with a small lookahead so the HWDGE queue
stays shallow and flushes don't block the ACT sequencer.

Cost model budget/core: DMA ~26us (x 23.3 + m 1.5 + rest), DVE ~24.5,
ACT ~25.4, PE ~21 -> DMA-bound.
"""

import sys
import numpy as np

sys.path.insert(0, "/opt/trn_rl_repo")

import ml_dtypes

import concourse.bacc as bacc  # noqa: E402
import concourse.tile as tile  # noqa: E402
from concourse import mybir  # noqa: E402

fp8 = ml_dtypes.float8_e4m3

F32 = mybir.dt.float32
F16 = mybir.dt.float16
F8 = mybir.dt.float8e4
f32 = np.float32
f16 = np.float16

B_TOT = 131072
IN = 256
HID = 128
H = 4
D = 32
NCORES = 8
THRESH = 0.7
BLK = 512
RPC = B_TOT // NCORES          # 16384 rows per core
NBLK = RPC // BLK              # 32 blocks of 512
NGRP = NBLK // 4               # 8 sm groups of 4 blocks (one PSUM bank each)
NPAIR = NBLK // 2              # 16 i-side pairs
SUPER = [512] * 32  # sums to RPC
WARMUP = 20
LOOKAHEAD = 12

# cnt columns: 0..31 per-block S_t (is_gt counts) | 32 S_i block0 | 33 S_i
# block1 | 34..48 i-pairs 1..15: sum(sign(v-thr)) over 1024 = 2*S_pair-1024
# | 49 C_t | 50 C_i
CNTW = 51

_cache = {}


def _build_kernel():
    nc = bacc.Bacc("TRN2", target_bir_lowering=False, debug=False)
    # x: single fp8 plane, both tensors, DR row-pair interleave:
    # row 4p+j: j in {0,1}: text K-rows (2p, 2p+1); j in {2,3}: image.
    x_all = nc.dram_tensor("x_all", (2 * IN, RPC), F8, kind="ExternalInput")
    # wall: [wtv_hi(256) | wiv_hi(256) | per window j in 0..3:
    #        (smt_hi_j, smt_e1_j, smi_hi_j, smi_e1_j) each 96 cols] DR-packed.
    # sm windows are 12 rows, densely packed at partitions 12j (the
    # partition-0 stacking has no quadrant alignment requirement).
    wall = nc.dram_tensor("wall", (128, 2048), F8, kind="ExternalInput")
    # aux cols: 0 thr_t (rho-compensated), 1 thr_i, 2 -thr_i, 3 unused
    aux = nc.dram_tensor("aux", (HID, 4), F32, kind="ExternalInput")
    # label row of block 0 (broadcast across partitions on device)
    plab = nc.dram_tensor("plab", (1, BLK), F16, kind="ExternalInput")
    # sm pre-activations (scaled 1/16, bias-free, fp16):
    # row 12j+k = slot k of window j (0:4 score_pre, 4:8 tvd, 8:12 ivd)
    m_out = nc.dram_tensor("m_out", (48, NGRP * BLK), F16,
                           kind="ExternalOutput")
    cnt = nc.dram_tensor("cnt", (HID, CNTW), F16, kind="ExternalOutput")

    x3 = x_all[:].rearrange("(p j) c -> p j c", j=4)
    sb_max = max(SUPER)

    with tile.TileContext(nc) as tc:
        with (
            nc.allow_low_precision(
                reason="counts <= 1024 are exact in fp16"),
            tc.tile_pool(name="w", bufs=1) as wp,
            tc.tile_pool(name="x", bufs=12) as xp,
            tc.tile_pool(name="fv", bufs=6) as fp,
            tc.tile_pool(name="r", bufs=3) as rp,
            tc.tile_pool(name="c", bufs=1) as cp,
            tc.tile_pool(name="ptv", bufs=2, space="PSUM") as ptvp,
            tc.tile_pool(name="piv", bufs=2, space="PSUM") as pivp,
            tc.tile_pool(name="psm", bufs=2, space="PSUM") as psmp,
        ):
            # ---- fill: weight wall first (first matmuls gate on it),
            # then x superblock 0
            wall_sb = wp.tile([128, 2048], F8, tag="wall")
            nc.sync.dma_start(wall_sb[:], wall[:])
            s0 = SUPER[0]
            xt0 = xp.tile([128, 4, sb_max], F8, tag="x")
            nc.sync.dma_start(xt0[:, :, :s0], x3[:, :, 0:s0])
            aux_sb = wp.tile([HID, 4], F32, tag="aux")
            nc.gpsimd.dma_start(aux_sb[:], aux[:])
            plab_row = wp.tile([1, BLK], F16, tag="plabr")
            nc.gpsimd.dma_start(plab_row[:], plab[:])
            thr_t = aux_sb[:, 0:1]
            thr_i = aux_sb[:, 1:2]
            nthr_i = aux_sb[:, 2:3]

            def wslc(a, b):
                return wall_sb[:, a:b].rearrange("p (i m) -> p i m", i=2)

            wtv_sb = wslc(0, 256)
            wiv_sb = wslc(256, 512)
            # [j][0]=smt_hi, [1]=smt_e1, [2]=smi_hi, [3]=smi_e1
            wsm_sb = [[wslc(512 + 384 * j + 96 * k,
                            512 + 384 * j + 96 * (k + 1))
                       for k in range(4)] for j in range(4)]

            # ---- PE p-state warmup chain during the DMA fill (squats the
            # first ptv pool buffer; WAW-serialized with block 0's tv)
            wuz = wp.tile([1, 128], F16, tag="wuz")
            nc.vector.memset(wuz[:], 0.0)
            one_sb = wp.tile([1, 128], F16, tag="one")
            nc.vector.memset(one_sb[:], 1.0)
            pwu = ptvp.tile([128, BLK], F32, tag="ptv")
            for _ in range(WARMUP):
                nc.tensor.matmul(pwu[0:128, 0:128], wuz[0:1, :], wuz[0:1, :],
                                 start=True, stop=True)
            # broadcast block 0's label row across partitions with a K=1
            # matmul into the second ptv buffer, then park it in SBUF; the
            # bank frees before block 1's tv matmul needs it
            plab_bc = ptvp.tile([128, BLK], F32, tag="ptv")
            nc.tensor.matmul(plab_bc[:], one_sb[0:1, :], plab_row[0:1, :],
                             start=True, stop=True)
            plab_sb = wp.tile([128, BLK], F16, tag="plabsb")
            nc.scalar.activation(plab_sb[:], plab_bc[:],
                                 mybir.ActivationFunctionType.Identity)

            # 1-col dummies trigger the one-time activation-table loads
            # (Sign lives outside the default set) during the DMA fill
            wu2 = wp.tile([1, 2], F32, tag="wu2")
            nc.scalar.activation(wu2[:, 0:1], wuz[0:1, 0:1],
                                 mybir.ActivationFunctionType.Sign)
            nc.scalar.activation(wu2[:, 1:2], wuz[0:1, 0:1],
                                 mybir.ActivationFunctionType.Identity)

            cnt_sb = cp.tile([HID, CNTW], F16, tag="cnt")

            DR = mybir.MatmulPerfMode.DoubleRow
            AF = mybir.ActivationFunctionType
            GT = mybir.AluOpType.is_gt
            ADD = mybir.AluOpType.add
            MUL = mybir.AluOpType.mult

            # block -> (superblock idx, col offset); superblock DMAs issued
            # on demand so the HWDGE queue stays shallow
            blk2sb = []
            sb_off = []
            off = 0
            for si, size in enumerate(SUPER):
                sb_off.append(off)
                for k in range(size // BLK):
                    blk2sb.append((si, k * BLK))
                off += size
            sb_tiles = {0: xt0}
            issued = [1]

            def ensure_issued(si):
                while issued[0] <= min(si, len(SUPER) - 1):
                    s = issued[0]
                    t = xp.tile([128, 4, sb_max], F8, tag="x")
                    nc.sync.dma_start(
                        t[:, :, :SUPER[s]],
                        x3[:, :, sb_off[s]:sb_off[s] + SUPER[s]])
                    sb_tiles[s] = t
                    issued[0] += 1

            def xslices(b):
                si, o = blk2sb[b]
                ensure_issued(si + LOOKAHEAD)
                xt = sb_tiles[si]
                return xt[:, 0:2, o:o + BLK], xt[:, 2:4, o:o + BLK]

            psm_t = None
            piv_t = None
            for b in range(NBLK):
                g, j = divmod(b, 4)
                pr, half = divmod(b, 2)
                xh, xi = xslices(b)
                if j == 0:
                    # all 16 sm matmuls of the group up front (one
                    # accumulation group), windows stacked via the
                    # zero-padded position stationaries
                    psm_t = psmp.tile([48, BLK], F32, tag="psm")
                    win = psm_t[:]
                    for jj in range(4):
                        xhj, xij = xslices(b + jj)
                        ws = wsm_sb[jj]
                        nc.tensor.matmul(win, ws[0], xhj, perf_mode=DR,
                                         start=(jj == 0), stop=False,
                                         skip_group_check=True)
                        nc.tensor.matmul(win, ws[1], xhj, perf_mode=DR,
                                         start=False, stop=False,
                                         skip_group_check=True)
                        nc.tensor.matmul(win, ws[2], xij, perf_mode=DR,
                                         start=False, stop=False,
                                         skip_group_check=True)
                        nc.tensor.matmul(win, ws[3], xij, perf_mode=DR,
                                         start=False, stop=(jj == 3),
                                         skip_group_check=True)
                ptv = ptvp.tile([128, BLK], F32, tag="ptv")
                nc.tensor.matmul(ptv[:], wtv_sb, xh, perf_mode=DR,
                                 start=True, stop=True)
                if half == 0:
                    piv_t = pivp.tile([128, 2 * BLK], F32, tag="piv")
                piv_h = piv_t[:, half * BLK:(half + 1) * BLK]
                nc.tensor.matmul(piv_h, wiv_sb, xi, perf_mode=DR,
                                 start=True, stop=True)

                # ---- t-side binarize + count (DVE, per block)
                fvt = fp.tile([128, BLK], F16, tag="fv16")
                nc.vector.tensor_scalar(
                    fvt[:], ptv[:], thr_t, None, op0=GT, op1=ADD,
                    accum_out=cnt_sb[:, b:b + 1])

                # ---- i-side binarize + count
                if b == 0:
                    fvi0 = fp.tile([128, BLK], F16, tag="fv16")
                    nc.vector.tensor_scalar(
                        fvi0[:], piv_h, thr_i, None, op0=GT, op1=ADD,
                        accum_out=cnt_sb[:, 32:33])
                    # label-weighted counts for the mixed block
                    fvl = fp.tile([128, BLK], F16, tag="fvl")
                    nc.vector.scalar_tensor_tensor(
                        fvl[:], fvt[:], 1.0, plab_sb[:], op0=MUL, op1=MUL,
                        accum_out=cnt_sb[:, 49:50])
                    nc.vector.scalar_tensor_tensor(
                        fvl[:], fvi0[:], 1.0, plab_sb[:], op0=MUL, op1=MUL,
                        accum_out=cnt_sb[:, 50:51])
                elif b == 1:
                    fvi1 = fp.tile([128, BLK], F16, tag="fv16")
                    nc.vector.tensor_scalar(
                        fvi1[:], piv_h, thr_i, None, op0=GT, op1=ADD,
                        accum_out=cnt_sb[:, 33:34])
                if j == 3:
                    # one identity moves the stacked sm bank to fp16 SBUF
                    # (scale 1/16; host adds bsm + sigmoid), then one plain
                    # 2-D HWDGE flush; emitted before the pair sign on the
                    # in-order ACT engine
                    rt = rp.tile([48, BLK], F16, tag="rt")
                    nc.scalar.activation(rt[:], psm_t[:], AF.Identity,
                                         scale=1.0 / 16.0)
                    nc.scalar.dma_start(m_out[:, g * BLK:(g + 1) * BLK],
                                        rt[:])

                if half == 1 and b > 1:
                    # one wide ACT Sign(+accum) per label-homogeneous pair
                    fvi = fp.tile([128, 2 * BLK], F16, tag="fvw")
                    nc.scalar.activation(
                        fvi[:], piv_t[:], AF.Sign, bias=nthr_i,
                        accum_out=cnt_sb[:, 33 + pr:34 + pr])

            nc.sync.dma_start(cnt[:], cnt_sb[:])

    nc.compile()
    return nc


def _get_kernels():
    if "a" not in _cache:
        _cache["a"] = _build_kernel()
    return (_cache["a"],)


class _Runner:
    """Persistent jitted SPMD executor for a compiled Bass module."""

    def __init__(self, nc):
        import jax
        from jax.sharding import Mesh, PartitionSpec
        from jax.experimental.shard_map import shard_map
        from concourse import bass2jax

        bass2jax.install_neuronx_cc_hook()
        self._nc = nc
        pname = nc.partition_id_tensor.name if nc.partition_id_tensor else None
        in_names, out_names, out_avals = [], [], []
        self._zero_outs = []
        for alloc in nc.m.functions[0].allocations:
            if not isinstance(alloc, mybir.MemoryLocationSet):
                continue
            nm = alloc.memorylocations[0].name
            if alloc.kind == "ExternalInput":
                if nm != pname:
                    in_names.append(nm)
            elif alloc.kind == "ExternalOutput":
                out_names.append(nm)
                shape = tuple(alloc.tensor_shape)
                dt = mybir.dt.np(alloc.dtype)
                out_avals.append(jax.core.ShapedArray(shape, dt))
                self._zero_outs.append(np.zeros(shape, dt))
        self._in_names = in_names
        self._out_names = out_names
        all_in_names = in_names + out_names + ([pname] if pname else [])

        def _body(*args):
            operands = list(args)
            if pname:
                operands.append(bass2jax.partition_id_tensor())
            outs = bass2jax._bass_exec_p.bind(
                *operands, out_avals=tuple(out_avals),
                in_names=tuple(all_in_names), out_names=tuple(out_names),
                lowering_input_output_aliases=(), sim_require_finite=True,
                sim_require_nnan=True, nc=nc)
            return tuple(outs)

        devices = jax.devices()[:NCORES]
        assert len(devices) == NCORES, f"need {NCORES} devices"
        mesh = Mesh(np.asarray(devices), ("core",))
        nio = len(in_names) + len(out_names)
        self._fn = jax.jit(
            shard_map(_body, mesh=mesh,
                      in_specs=(PartitionSpec("core"),) * nio,
                      out_specs=(PartitionSpec("core"),) * len(out_names),
                      check_rep=False),
            keep_unused=True)

    def __call__(self, in_maps):
        assert len(in_maps) == NCORES
        concat = [
            np.concatenate([np.asarray(m[n]) for m in in_maps], axis=0)
            for n in self._in_names
        ]
        concat += [
            np.zeros((NCORES * z.shape[0], *z.shape[1:]), z.dtype)
            for z in self._zero_outs
        ]
        out_arrs = self._fn(*concat)
        results = []
        for c in range(NCORES):
            d = {}
            for i, nm in enumerate(self._out_names):
                full = np.asarray(out_arrs[i])
                per = full.shape[0] // NCORES
                d[nm] = full[c * per:(c + 1) * per]
            results.append(d)
        return results


def _get_runner():
    if "ra" not in _cache:
        (nc_a,) = _get_kernels()
        _cache["ra"] = _Runner(nc_a)
    return _cache["ra"]


def _fold_params(p):
    """Fold params into device weights + host base projection (f64 math)."""
    Wout = p["Wout"].astype(np.float64)
    bout = p["bout"].astype(np.float64)
    attn_W = p["attn_W"].astype(np.float64)
    attn_b = p["attn_b"].astype(np.float64)
    W1 = Wout[0, :HID]
    W2 = Wout[0, HID:2 * HID]
    W3 = Wout[0, 2 * HID:]

    A_t = np.zeros((HID, H))
    A_i = np.zeros((HID, H))
    Bt = np.zeros((HID, H))
    for h in range(H):
        A_t[h * D:(h + 1) * D, h] = attn_W[h, :D]
        A_i[h * D:(h + 1) * D, h] = attn_W[h, D:]
        Bt[h * D:(h + 1) * D, h] = W1[h * D:(h + 1) * D]

    def WT(name):
        return p[name].astype(np.float64).T  # (IN, HID)

    # sm slots: [score_pre(4) | tvd(4) | ivd(4)] -> 12 per 16-row window
    wsmt = np.zeros((IN, 12))
    wsmt[:, 0:4] = WT("Wtq") @ A_t
    wsmt[:, 4:8] = WT("Wtv") @ Bt
    wsmi = np.zeros((IN, 12))
    wsmi[:, 0:4] = WT("Wik") @ A_i
    wsmi[:, 8:12] = WT("Wiv") @ Bt

    bsm = np.zeros(12)
    bsm[0:4] = (p["btq"].astype(np.float64) @ A_t
                + p["bik"].astype(np.float64) @ A_i + attn_b)
    bsm[4:8] = p["btv"].astype(np.float64) @ Bt
    bsm[8:12] = p["biv"].astype(np.float64) @ Bt

    def dr_pack(Wv):
        # DoubleRow stationary layout [p, (i m)]: col i*M+m = Wv[2p+i, m]
        M = Wv.shape[1]
        g = np.empty((128, 2 * M), dtype=fp8)
        g[:, 0:M] = Wv[0::2]
        g[:, M:] = Wv[1::2]
        return g

    def hi_e1(W):
        W16 = (16.0 * W).astype(f32)
        hi = W16.astype(fp8)
        e1 = (W16 - hi.astype(f32)).astype(fp8)
        return hi, e1

    wtv_hi = (16.0 * WT("Wtv")).astype(f32).astype(fp8)
    wiv_hi = (16.0 * WT("Wiv")).astype(f32).astype(fp8)
    smt_hi, smt_e1 = hi_e1(wsmt)
    smi_hi, smi_e1 = hi_e1(wsmi)

    wall = np.zeros((128, 2048), dtype=fp8)
    wall[:, 0:256] = dr_pack(wtv_hi)
    wall[:, 256:512] = dr_pack(wiv_hi)
    for j in range(4):
        for k, w in enumerate((smt_hi, smt_e1, smi_hi, smi_e1)):
            wj = np.zeros((IN, 48), dtype=fp8)
            wj[:, 12 * j:12 * j + 12] = w
            base = 512 + 384 * j + 96 * k
            wall[:, base:base + 96] = dr_pack(wj)

    # thresholds vs 16x-scaled hi-only products, rho-compensated for the
    # weight quantization's systematic per-feature component
    def rho(W):
        W16 = 16.0 * W
        hi = W16.astype(f32).astype(fp8).astype(np.float64)
        return (hi * W16).sum(0) / (W16 * W16).sum(0)

    aux = np.zeros((HID, 4), dtype=f32)
    aux[:, 0] = (16.0 * (THRESH - p["btv"].astype(np.float64))
                 * rho(WT("Wtv"))).astype(f32)
    aux[:, 1] = (16.0 * (THRESH - p["biv"].astype(np.float64))
                 * rho(WT("Wiv"))).astype(f32)
    aux[:, 2] = -aux[:, 1]

    # host base projection: Wout . [t_Q, i_Q] + bout  (exact, f32)
    w12_t = (WT("Wtq") @ W2).astype(f32)
    w12_i = (WT("Wiq") @ W3).astype(f32)
    b12 = f32(p["btq"].astype(np.float64) @ W2
              + p["biq"].astype(np.float64) @ W3 + bout[0])

    dev = {"wall": wall, "aux": aux}
    return dev, w12_t, w12_i, b12, bsm.astype(f32)


def _chi_square_from_counts(S, C, L, B):
    F = S.shape[0]
    counts = np.zeros((F, 2, 2), dtype=f32)
    counts[:, 1, 1] = C
    counts[:, 1, 0] = S - C
    counts[:, 0, 1] = L - C
    counts[:, 0, 0] = B - S - L + C
    total = counts.sum(axis=(1, 2), dtype=f32)
    col = counts.sum(axis=1, dtype=f32)
    row = counts.sum(axis=2, dtype=f32)
    expected = col[:, :, None] * row[:, None, :] / (total[:, None, None] + f32(1e-6))
    chi = ((counts - expected) ** 2 / (expected + f32(1e-6))).sum(
        axis=(1, 2), dtype=f32)
    return chi


def _x_pack(xt, xi):
    """(256, n) f32 x2 -> (512, n) fp8: row 4p+j = {j<2: text, j>=2: image}
    K-row 2p+(j%2) (DoubleRow row-pair interleave)."""
    n = xt.shape[1]
    packed = np.empty((512, n), dtype=fp8)
    pv = packed.reshape(128, 4, n)
    pv[:, 0:2] = xt.astype(fp8).reshape(128, 2, n)
    pv[:, 2:4] = xi.astype(fp8).reshape(128, 2, n)
    return packed


def _core_permutation(label, chunk):
    """Order chunk rows as [mixed 512-block | parity filler | pure0 blocks |
    pure1 blocks] so that i-side PAIRS (1..15) are label-homogeneous.

    Returns (src, pure1_blocks) where pure1_blocks marks blocks 1..31 that
    are all-label-1.
    """
    lab = (label[chunk] != 0)
    n0 = int((~lab).sum())
    k0, r0 = divmod(n0, BLK)
    zeros = chunk[~lab]
    ones = chunk[lab]
    if r0 > 0:
        mixed = np.concatenate([zeros[k0 * BLK:], ones[0:BLK - r0]])
        zeros = zeros[0:k0 * BLK]
        ones = ones[BLK - r0:]
    else:
        # no mixed block: use a pure block (zeros if any) as "mixed"
        if k0 > 0:
            mixed = zeros[0:BLK]
            zeros = zeros[BLK:]
        else:
            mixed = ones[0:BLK]
            ones = ones[BLK:]
    n1b = len(ones) // BLK
    assert len(ones) % BLK == 0 and len(zeros) % BLK == 0
    if n1b % 2 == 1:
        filler = ones[0:BLK]
        ones = ones[BLK:]
    else:
        if len(zeros) >= BLK:
            filler = zeros[0:BLK]
            zeros = zeros[BLK:]
        else:
            # degenerate: all-ones core; n1b even, take two ones blocks
            filler = ones[0:BLK]
            ones = ones[BLK:]
            # pairs stay homogeneous (everything is ones)
    src = np.concatenate([mixed, filler, zeros, ones])
    lab_perm = (label[src] != 0)
    blocks = lab_perm.reshape(NBLK, BLK)
    pure1 = blocks.all(axis=1)
    mixed_mask = blocks.any(axis=1) & ~pure1
    assert not mixed_mask[1:].any(), "only block 0 may be mixed"
    # pairs 1..15 must be homogeneous
    pb = pure1[2:].reshape(NPAIR - 1, 2)
    assert (pb[:, 0] == pb[:, 1]).all(), "pairs must be label-homogeneous"
    return src, pure1


def kernel(**inputs):
    text = np.asarray(inputs["text_vec"], dtype=f32)
    image = np.asarray(inputs["image_vec"], dtype=f32)
    label = np.asarray(inputs["label"]).astype(np.int64)

    dev_w, w12_t, w12_i, b12, bsm12 = _fold_params(inputs)
    run = _get_runner()

    # host base projection (exact skip term)
    base_all = (text @ w12_t + image @ w12_i + b12).astype(f32)

    order = np.concatenate([np.flatnonzero(label == 0),
                            np.flatnonzero(label != 0)])
    in_maps = []
    srcs = []
    pure1s = []
    for c in range(NCORES):
        chunk = order[c * RPC:(c + 1) * RPC]
        src, pure1 = _core_permutation(label, chunk)
        lab_perm = (label[src] != 0)
        m = {
            "x_all": _x_pack(np.ascontiguousarray(text[src].T),
                             np.ascontiguousarray(image[src].T)),
            "plab": np.ascontiguousarray(
                lab_perm[:BLK].astype(f16).reshape(1, BLK)),
        }
        m.update(dev_w)
        in_maps.append(m)
        srcs.append(src)
        pure1s.append(pure1)

    res = run(in_maps)

    # ---- host: reduce count tables, compute alpha (tiny "all-reduce")
    S_t = np.zeros(HID)
    S_i = np.zeros(HID)
    C_t = np.zeros(HID)
    C_i = np.zeros(HID)
    for c in range(NCORES):
        cc = res[c]["cnt"].astype(np.float64)
        st = cc[:, 0:32]
        si01 = cc[:, 32:34]                     # blocks 0,1 (is_gt counts)
        sip = (cc[:, 34:49] + 2 * BLK) * 0.5    # pairs 1..15 (sign-encoded)
        S_t += st.sum(axis=1)
        S_i += si01.sum(axis=1) + sip.sum(axis=1)
        p1 = pure1s[c]
        C_t += st[:, 1:][:, p1[1:]].sum(axis=1) + cc[:, 49]
        if p1[1]:
            C_i += si01[:, 1]
        p1pair = p1[2::2]                       # pairs 1..15 homogeneity
        C_i += sip[:, p1pair].sum(axis=1) + cc[:, 50]
    L = float((label != 0).sum())
    chi_t = _chi_square_from_counts(S_t, C_t, L, float(B_TOT))
    chi_i = _chi_square_from_counts(S_i, C_i, L, float(B_TOT))
    chi_max = f32(max(chi_t.max(), chi_i.max()))
    alpha_t = (chi_t / (chi_max + f32(1e-6)))[:H].astype(f32)
    alpha_i = (chi_i / (chi_max + f32(1e-6)))[:H].astype(f32)
    atai = -(alpha_t * alpha_i)

    # ---- host: alpha combination + un-permutation
    out = np.empty((B_TOT, 1), dtype=f32)
    for c in range(NCORES):
        # m_out (48, 4096): row 12j+k = slot k of window j, col 512g+cc;
        # fp16 of v/16, bias-free
        mm = res[c]["m_out"].astype(f32).reshape(4, 12, NGRP, BLK)
        # -> [g, j, cc, k]: batch row = 512*(4g+j) + cc
        sm = mm.transpose(2, 0, 3, 1).reshape(RPC, 12) + bsm12[None, :]
        s = 1.0 / (1.0 + np.exp(-sm[:, 0:4], dtype=f32))
        tvd = sm[:, 4:8]
        ivd = sm[:, 8:12]
        u = s * tvd
        v = s * ivd
        out_pc = (base_all[srcs[c]] + u @ alpha_t + v @ alpha_i
                  + (s * v) @ atai)
        out[srcs[c], 0] = out_pc
    return out
